# revision 21
# baseline (speedup 1.0000x reference)
"""Trainium2 Bass kernel for GQA multi-head attention (TP-8 over heads).

Problem: hidden [1, 4096, 2048] fp32; wq [2048, 2048], wk/wv [2048, 512],
wo [2048, 2048]; 16 q-heads / 4 kv-heads, head_dim 128, interleaved RoPE,
causal softmax attention, o_proj.

Sharding: core c in 0..7 handles q-heads {2c, 2c+1} and kv-head c//2
(kv proj duplicated across core pairs). Each core produces a partial
o_proj output [4096, 2048] (fp16); the host sums the 8 partials in fp32.

v5 structure (trace-driven):
- hidden transposed on the HOST -> plain DMAs (no transpose-queue limit).
- prologue split across queues (weights on gpsimd, hT on sync) + PE
  warmup matmuls so HAM is hot and PE starts ~8us in.
- fused loop: proj(i) | deferred PV tail(i-1) | finalize(i-1) |
  o_proj(i-1) | attention(i) with scores issued 2 k-tiles ahead.
- [128,1024] fp32 score tiles; ONE exp per k-tile, causally restricted.
- diagonal k-tiles: PV split into mask-free live part + masked triangle.
- PSUM: pq ring 2 banks | shared sc/po/dps/tp ring 2x2 banks | acc 2.
"""

import sys

sys.path.insert(0, "/opt/trn_rl_repo")

import math

import numpy as np

NUM_HEADS = 16
NUM_KV = 4
HD = 128
H = 2048
KVD = 512
ROPE_BASE = 10000.0
S_FULL = 4096
N_CORES = 8


def _rope_tables(S):
    inv = 1.0 / (ROPE_BASE ** (np.arange(0, HD, 2, dtype=np.float64) / HD))
    t = np.arange(S, dtype=np.float64)
    fr = t[:, None] * inv[None, :]  # [S, 64]
    cos = np.repeat(np.cos(fr), 2, axis=1)
    sin = np.repeat(np.sin(fr), 2, axis=1)
    sin2 = sin.copy()
    sin2[:, 0::2] *= -1.0  # even dims get -sin (r1 = x1*cos - x2*sin)
    return cos, sin2


def _rope_tables_v4(S):
    """cos3 [S, 384]; sinE3/sinO3 [S, 192] for the fused strided swap-muls.

    rot[2i]   = x[2i]*cos[2i]     + x[2i+1]*sin2[2i]   (sin2[2i] = -sin)
    rot[2i+1] = x[2i+1]*cos[2i+1] + x[2i]*sin2[2i+1]   (sin2[2i+1] = +sin)
    """
    import ml_dtypes

    cos, sin2 = _rope_tables(S)
    cos3 = np.tile(cos, (1, 3)).astype(ml_dtypes.bfloat16)
    sinE3 = np.tile(sin2[:, 0::2], (1, 3)).astype(ml_dtypes.bfloat16)
    sinO3 = np.tile(sin2[:, 1::2], (1, 3)).astype(ml_dtypes.bfloat16)
    return cos3, sinE3, sinO3


def build(S=S_FULL):
    import ml_dtypes

    import concourse.bacc as bacc
    import concourse.mybir as mybir
    import concourse.tile as tile

    f32 = mybir.dt.float32
    bf16 = mybir.dt.bfloat16
    f16 = mybir.dt.float16
    AF = mybir.ActivationFunctionType
    ALU = mybir.AluOpType

    NCH = S // 512
    NT = S // 128
    KT = H // 128
    scale = 1.0 / math.sqrt(HD)

    nc = bacc.Bacc("TRN2", target_bir_lowering=False, debug=False, num_devices=N_CORES)

    hidT = nc.dram_tensor("hidT", [H, S], bf16, kind="ExternalInput")
    wqkv = nc.dram_tensor("wqkv_s", [H, 512], bf16, kind="ExternalInput")
    wo = nc.dram_tensor("wo_s", [2 * HD, H], bf16, kind="ExternalInput")
    out = nc.dram_tensor("out_part", [S, H], f16, kind="ExternalOutput")

    cos3_np, sinE3_np, sinO3_np = _rope_tables_v4(S)
    cos_d = nc.inline_tensor(cos3_np, name="cos_tab")
    sinE_d = nc.inline_tensor(sinE3_np, name="sinE_tab")
    sinO_d = nc.inline_tensor(sinO3_np, name="sinO_tab")
    ident_d = nc.inline_tensor(np.eye(128, dtype=ml_dtypes.bfloat16), name="ident")

    with tile.TileContext(nc) as tc:
        with tc.tile_pool(name="pers", bufs=1) as pers:
            qt0 = pers.tile([128, S], bf16, tag="qt0")
            qt1 = pers.tile([128, S], bf16, tag="qt1")
            kt = pers.tile([128, S], bf16, tag="kt")
            vnat = pers.tile([128, NT, HD], bf16, tag="vnat")
            wcat = pers.tile([128, KT, 512], bf16, tag="wcat")
            wo_sb = pers.tile([128, 2, H], bf16, tag="wo")
            ones_m = pers.tile([128, 128], f16, tag="ones_m")
            wrm = pers.tile([128, 512], f16, tag="wrm")
            ident_sb = pers.tile([128, 128], bf16, tag="ident")
            nc.vector.memset(ones_m[:], 1.0)
            nc.vector.memset(wrm[:], 0.0)
            # weights k-streamed on the scalar queue (parallel with hT on
            # sync) so the first k-loop starts as soon as slice 0 lands
            nc.gpsimd.dma_start(ident_sb[:], ident_d.ap())
            for q4 in range(0, KT, 4):
                nc.scalar.dma_start(
                    wcat[:, q4 : q4 + 4, :],
                    wqkv.ap()[128 * q4 : 128 * (q4 + 4), :].rearrange(
                        "(T p) m -> p T m", p=128
                    ),
                )

            with (
                tc.tile_pool(name="hT", bufs=2) as hTp,
                tc.tile_pool(name="trig", bufs=2) as trigp,
                tc.tile_pool(name="rope", bufs=3) as ropep,
                tc.tile_pool(name="pt", bufs=6) as ptp,
                tc.tile_pool(name="den", bufs=2) as denp,
                tc.tile_pool(name="fin", bufs=2) as finp,
                tc.tile_pool(name="ost", bufs=3) as ostp,
                tc.tile_pool(name="ps1", bufs=2, space="PSUM") as ps1,
                tc.tile_pool(name="psw", bufs=2, space="PSUM") as psw,
                tc.tile_pool(name="psacc", bufs=2, space="PSUM") as psacc,
            ):
                # ---- prologue: hT chunk 0 k-streamed on the sync queue ----
                hts = [None, None]  # per-chunk parity
                hts[0] = hTp.tile([128, KT, 512], bf16, tag="hT", name="hT_0")
                for kq in range(0, KT, 2):
                    nc.sync.dma_start(
                        hts[0][:, kq : kq + 2, :],
                        hidT.ap()[128 * kq : 128 * (kq + 2), 0:512].rearrange(
                            "(T p) m -> p T m", p=128
                        ),
                    )

                def load_trig(ch):
                    tg = trigp.tile([128, 4, 768], bf16, tag="trig", name=f"trig_{ch}")
                    r0 = 512 * ch
                    nc.gpsimd.dma_start(
                        tg[:, :, 0:384],
                        cos_d.ap()[r0 : r0 + 512, :].rearrange("(q p) d -> p q d", p=128),
                    )
                    nc.gpsimd.dma_start(
                        tg[:, :, 384:576],
                        sinE_d.ap()[r0 : r0 + 512, :].rearrange("(q p) d -> p q d", p=128),
                    )
                    nc.gpsimd.dma_start(
                        tg[:, :, 576:768],
                        sinO_d.ap()[r0 : r0 + 512, :].rearrange("(q p) d -> p q d", p=128),
                    )
                    return tg

                trig = [None, None]
                trig[0] = load_trig(0)

                # PE warmup: keep HAM hot + occupy PE during input DMAs
                # (cheap N=128 matmuls so overshoot past DMA-ready is small)
                warm = ps1.tile([128, 512], f32, tag="pq", name="warm")
                for _ in range(100):
                    nc.tensor.matmul(
                        warm[:, 0:128], ones_m[:], wrm[:, 0:128],
                        start=True, stop=True,
                    )

                # ---- chunk state ----
                state = {}  # i -> dict(acc_h, den, nk)
                an_of = {}  # i -> [an0, an1]
                pend_pv = []  # deferred (issue_pv, kk) from previous chunk
                drain_flip = 0

                def emit_proj_tile(i, t):
                    par = i % 2
                    g = 4 * i + t
                    pq = ps1.tile([128, 512], f32, tag="pq", name=f"pq_{g}")
                    for k in range(KT):
                        nc.tensor.matmul(
                            pq[:],
                            hts[par][:, k, 128 * t : 128 * (t + 1)],
                            wcat[:, k, :],
                            start=(k == 0),
                            stop=(k == KT - 1),
                        )
                    nat = ropep.tile([128, 512], bf16, tag="nat")
                    nc.scalar.copy(nat[:], pq[:])
                    nc.vector.tensor_copy(vnat[:, g, :], nat[:, 384:512])
                    natp = nat[:, 0:384].rearrange("p (i two) -> p two i", two=2)
                    xsw = ropep.tile([128, 384], bf16, tag="xsw")
                    xwv = xsw[:].rearrange("p (i two) -> p two i", two=2)
                    nc.vector.tensor_mul(
                        xwv[:, 0, :], natp[:, 1, :], trig[par][:, t, 384:576]
                    )
                    nc.vector.tensor_mul(
                        xwv[:, 1, :], natp[:, 0, :], trig[par][:, t, 576:768]
                    )
                    rot = ropep.tile([128, 384], bf16, tag="rot")
                    nc.vector.tensor_mul(rot[:], nat[:, 0:384], trig[par][:, t, 0:384])
                    nc.vector.tensor_add(rot[:], rot[:], xsw[:])
                    tp = psw.tile([128, 384], bf16, tag="sc", name=f"tp_{g}")
                    for j in range(3):
                        nc.tensor.transpose(
                            tp[:, 128 * j : 128 * (j + 1)],
                            rot[:, 128 * j : 128 * (j + 1)],
                            ident_sb[:],
                        )
                    nc.vector.tensor_copy(qt0[:, 128 * g : 128 * (g + 1)], tp[:, 0:128])
                    nc.vector.tensor_copy(qt1[:, 128 * g : 128 * (g + 1)], tp[:, 128:256])
                    nc.vector.tensor_copy(kt[:, 128 * g : 128 * (g + 1)], tp[:, 256:384])

                def emit_finalize(i):
                    st = state.pop(i)
                    ans = []
                    for h in range(2):
                        dps = psw.tile([128, 512], f32, tag="sc", name=f"dps_{i}_{h}")
                        nc.tensor.matmul(
                            dps[:], ones_m[:],
                            st["den"][:, 512 * h : 512 * (h + 1)],
                            start=True, stop=(st["denb"] is None),
                        )
                        if st["denb"] is not None:
                            nc.tensor.matmul(
                                dps[:], ones_m[:],
                                st["denb"][:, 512 * h : 512 * (h + 1)],
                                start=False, stop=True,
                                skip_group_check=True,
                            )
                        au = finp.tile([128, 512], bf16, tag="au")
                        nc.scalar.copy(au[:], st["acc_h"][h])
                        rcb = finp.tile([128, 512], f32, tag="rc")
                        nc.vector.reciprocal_approx_fast(rcb[:], dps[:])
                        an = finp.tile([128, 512], bf16, tag="atn")
                        nc.vector.tensor_mul(an[:], au[:], rcb[:])
                        ans.append(an)
                    an_of[i] = ans

                def emit_oproj(i):
                    nonlocal drain_flip
                    ans = an_of.pop(i)
                    for t in range(4):
                        g = 4 * i + t
                        ost = ostp.tile([128, H], f16, tag="ost")
                        for nn in range(2):
                            po = psw.tile(
                                [128, 1024], f32, tag="sc", name=f"po_{g}_{nn}"
                            )
                            for sub in range(2):
                                for h in range(2):
                                    nc.tensor.matmul(
                                        po[:, 512 * sub : 512 * (sub + 1)],
                                        ans[h][:, 128 * t : 128 * (t + 1)],
                                        wo_sb[
                                            :, h,
                                            1024 * nn + 512 * sub
                                            : 1024 * nn + 512 * (sub + 1),
                                        ],
                                        start=(h == 0),
                                        stop=(h == 1),
                                        skip_group_check=True,
                                    )
                            if drain_flip % 2 == 0:
                                nc.vector.tensor_copy(
                                    ost[:, 1024 * nn : 1024 * (nn + 1)], po[:]
                                )
                            else:
                                nc.scalar.copy(
                                    ost[:, 1024 * nn : 1024 * (nn + 1)], po[:]
                                )
                            drain_flip += 1
                        oeng = nc.sync if t % 2 == 0 else nc.scalar
                        oeng.dma_start(out.ap()[128 * g : 128 * (g + 1), :], ost[:])

                def emit_attention(i):
                    acc_h = [
                        psacc.tile([128, 512], f32, tag="acc", name=f"acc_{i}_{h}")
                        for h in range(2)
                    ]
                    den = denp.tile([128, 1024], f16, tag="den", name=f"den_{i}")
                    denb = (
                        denp.tile([128, 1024], f16, tag="denb", name=f"denb_{i}")
                        if i > 0
                        else None
                    )
                    nk = 4 * (i + 1)
                    state[i] = {"acc_h": acc_h, "den": den, "denb": denb, "nk": nk}
                    qs0 = qt0[:, 512 * i : 512 * (i + 1)]
                    qs1 = qt1[:, 512 * i : 512 * (i + 1)]
                    pt_t = [None] * nk

                    def issue_sc(kk):
                        j = kk - 4 * i
                        off = 128 * j if j > 0 else 0
                        sc = psw.tile([128, 1024], f32, tag="sc", name=f"sc_{i}_{kk}")
                        ktile = kt[:, 128 * kk : 128 * (kk + 1)]
                        nc.tensor.matmul(
                            sc[:, off:512], ktile, qs0[:, off:512],
                            start=True, stop=True,
                        )
                        nc.tensor.matmul(
                            sc[:, 512 + off : 1024], ktile, qs1[:, off:512],
                            start=True, stop=True,
                        )
                        pt = ptp.tile([128, 1024], bf16, tag="pt", name=f"pt_{i}_{kk}")
                        if off:
                            src = sc[:].rearrange("p (s m) -> p s m", s=2)[:, :, off:512]
                            dst = pt[:].rearrange("p (s m) -> p s m", s=2)[:, :, off:512]
                            nc.scalar.activation(dst, src, AF.Exp, scale=scale)
                        else:
                            nc.scalar.activation(pt[:], sc[:], AF.Exp, scale=scale)
                        pt_t[kk] = pt

                    def issue_pv(kk):
                        j = kk - 4 * i
                        pt = pt_t[kk]
                        vtile = vnat[:, kk, :]
                        first = kk == 0
                        last = kk == nk - 1
                        if j >= 0:
                            tri = pt[:].rearrange("p (s m) -> p s m", s=2)[
                                :, :, 128 * j : 128 * (j + 1)
                            ]
                            nc.gpsimd.affine_select(
                                tri, tri, [[0, 2], [1, 128]], ALU.is_ge, 0.0,
                                base=0, channel_multiplier=-1,
                            )
                            lo = 128 * (j + 1)
                            for h in range(2):
                                if lo < 512:
                                    nc.tensor.matmul(
                                        acc_h[h][:, lo:512], vtile,
                                        pt[:, 512 * h + lo : 512 * (h + 1)],
                                        start=first, stop=False,
                                        skip_group_check=True,
                                    )
                                nc.tensor.matmul(
                                    acc_h[h][:, 128 * j : lo], vtile,
                                    pt[:, 512 * h + 128 * j : 512 * h + lo],
                                    start=False, stop=last,
                                    skip_group_check=True,
                                )
                            if first:
                                nc.vector.tensor_copy(den[:], pt[:])
                            else:
                                dsl = den[:].rearrange("p (s m) -> p s m", s=2)[
                                    :, :, 128 * j : 512
                                ]
                                psl = pt[:].rearrange("p (s m) -> p s m", s=2)[
                                    :, :, 128 * j : 512
                                ]
                                nc.vector.tensor_add(dsl, dsl, psl)
                        else:
                            for h in range(2):
                                nc.tensor.matmul(
                                    acc_h[h], vtile,
                                    pt[:, 512 * h : 512 * (h + 1)],
                                    start=first, stop=last,
                                    skip_group_check=True,
                                )
                            if first:
                                nc.vector.tensor_copy(den[:], pt[:])
                            elif kk == 1:
                                nc.gpsimd.tensor_copy(denb[:], pt[:])
                            elif kk % 2 == 1:
                                # independent accumulator on gpsimd
                                nc.gpsimd.tensor_add(denb[:], denb[:], pt[:])
                            else:
                                nc.vector.tensor_add(den[:], den[:], pt[:])

                    issue_sc(0)
                    if nk > 1:
                        issue_sc(1)
                    for kk in range(nk):
                        if kk + 2 < nk:
                            issue_sc(kk + 2)
                        if kk < nk - 2:
                            issue_pv(kk)
                        else:
                            pend_pv.append(lambda kk=kk, f=issue_pv: f(kk))

                # ---- main loop ----
                for i in range(NCH):
                    emit_proj_tile(i, 0)
                    # deferred PV tail of chunk i-1 (exp/mask ran under proj t0)
                    for f in pend_pv:
                        f()
                    pend_pv = []
                    # prefetch hT for chunk i+1 (after chunk i's critical DMAs)
                    if i + 1 < NCH:
                        np_ = (i + 1) % 2
                        hts[np_] = hTp.tile(
                            [128, KT, 512], bf16, tag="hT", name=f"hT_{i + 1}"
                        )
                        nc.sync.dma_start(
                            hts[np_][:],
                            hidT.ap()[:, 512 * (i + 1) : 512 * (i + 2)].rearrange(
                                "(T p) m -> p T m", p=128
                            ),
                        )
                        # trig prefetch after the deferred masks (gpsimd queue)
                        trig[(i + 1) % 2] = load_trig(i + 1)
                    if i == 0:
                        # wo needed first at o_proj(0) in body 1
                        nc.gpsimd.dma_start(
                            wo_sb[:], wo.ap().rearrange("(T p) m -> p T m", p=128)
                        )
                    for t in range(1, 4):
                        emit_proj_tile(i, t)
                    if i > 0:
                        emit_finalize(i - 1)
                        emit_oproj(i - 1)
                    emit_attention(i)

                # ---- epilogue ----
                for f in pend_pv:
                    f()
                pend_pv = []
                emit_finalize(NCH - 1)
                emit_oproj(NCH - 1)

    nc.compile()
    return nc


_CACHE = {}


def _get_program(S=S_FULL):
    if S not in _CACHE:
        _CACHE[S] = build(S)
    return _CACHE[S]


def shard_inputs(hidden_states, wq, wk, wv, wo):
    import ml_dtypes

    bf = ml_dtypes.bfloat16
    hidden_states = np.asarray(hidden_states)
    wq = np.asarray(wq)
    wk = np.asarray(wk)
    wv = np.asarray(wv)
    wo = np.asarray(wo)
    S = hidden_states.shape[1]
    hidT = np.ascontiguousarray(hidden_states.reshape(S, H).astype(bf).T)
    wqb = wq.astype(bf)
    wkb = wk.astype(bf)
    wvb = wv.astype(bf)
    wob = wo.astype(bf)
    in_maps = []
    for c in range(N_CORES):
        g = c // 2
        wqkv = np.ascontiguousarray(
            np.concatenate(
                [
                    wqb[:, 256 * c : 256 * (c + 1)],
                    wkb[:, 128 * g : 128 * (g + 1)],
                    wvb[:, 128 * g : 128 * (g + 1)],
                ],
                axis=1,
            )
        )
        in_maps.append(
            {
                "hidT": hidT,
                "wqkv_s": wqkv,
                "wo_s": np.ascontiguousarray(wob[256 * c : 256 * (c + 1), :]),
            }
        )
    return in_maps


def kernel(hidden_states, wq, wk, wv, wo, _trace=False):
    from concourse import bass_utils

    B, S, _ = hidden_states.shape
    nc = _get_program(S)
    in_maps = shard_inputs(hidden_states, wq, wk, wv, wo)
    res = bass_utils.run_bass_kernel_spmd(
        nc, in_maps, core_ids=list(range(N_CORES)), trace=_trace
    )
    acc = np.zeros((S, H), dtype=np.float32)
    for c in range(N_CORES):
        acc += res.results[c]["out_part"].astype(np.float32)
    out = acc.reshape(B, S, H)
    if _trace:
        return out, res
    return out
